# revision 19
# baseline (speedup 1.0000x reference)
"""MinibatchDiscrimination kernel for Trainium2 (8 NeuronCores, SPMD).

Math: Ms = (x @ W).reshape(B, 128, 16)
      norm[b,i,j] = sum_d |Ms[b,i,d] - Ms[b,j,d]|
      out[b,i]    = sum_j exp(-norm[b,i,j])

Sharding: data-parallel over batch B across 8 cores (256 samples each).
W replicated. Host pre-casts to bf16 and pre-transposes x so the
stationary matmul operand loads without an on-device transpose.

Engine assignment (per 128-sample tile, partitions = batch):
  - PE: Ms = x @ W in 4 col-chunks (descending), both tiles per chunk,
    8 PSUM banks; W streamed per chunk and freed.
  - DVE: broadcast subtract (bf16 2x), tree L1 (bf16 2x), per-block
    row-sum reduce and mirrored (transposed) reduce.
  - ACT: |.| in place on the diff, exp(-norm) -> bf16.
  - Pool (gpsimd): tree L2/L3/L4, PSUM->SBUF Ms copies, esum final
    reduction tree, memsets.
Symmetry: for j-block [j0, j0+8) compute only i in [j0, 128); mirrored
contributions for i >= j0+8 are added to O[j] via a transposed reduce.
Blocks with ni > 64 split into two i-chunks (SBUF + drain latency).
"""

import os
import sys

sys.path.insert(0, "/opt/trn_rl_repo")
os.environ.setdefault("MYCRO_LOCAL_CACHE", "1")

import numpy as np
from ml_dtypes import bfloat16

import concourse.bacc as bacc
import concourse.bass as bass
import concourse.tile as tile
from concourse import mybir
from concourse.bass_utils import run_bass_kernel_spmd

B, F, K, D = 2048, 2048, 128, 16
NCORES = 8
BL = B // NCORES          # 256 rows per core
P = 128                   # partitions
NBT = BL // P             # 2 batch tiles per core
FB = F // P               # 16 contraction blocks
ND = K * D                # 2048 output cols of the matmul
JB = 8                    # j-block width
NJB = K // JB             # 16 j-blocks

_BF16 = mybir.dt.bfloat16
_F32 = mybir.dt.float32
_AX = mybir.AxisListType.X
_ADD = mybir.AluOpType.add


def _build_nc():
    nc = bacc.Bacc("TRN2", target_bir_lowering=False, debug=False)
    xt = nc.dram_tensor("xt", [F, BL], _BF16, kind="ExternalInput")
    w = nc.dram_tensor("w", [F, ND], _BF16, kind="ExternalInput")
    out = nc.dram_tensor("out", [BL, K], _F32, kind="ExternalOutput")

    with tile.TileContext(nc) as tc:
        with (
            tc.tile_pool(name="const", bufs=1) as const_pool,
            tc.tile_pool(name="wstream", bufs=2) as wpool,
            tc.tile_pool(name="work", bufs=3) as work,
            tc.tile_pool(name="mid", bufs=2) as mid,
            tc.tile_pool(name="acc", bufs=1) as acc,
            tc.tile_pool(name="psum", bufs=1, space="PSUM") as psum_pool,
        ):
            xt_sb = const_pool.tile([P, FB, BL], _BF16)
            xt_r = xt.rearrange("(fb p) b -> p fb b", p=P)
            nc.scalar.dma_start(out=xt_sb, in_=xt_r)
            w_r = w.rearrange("(fb p) n -> p fb n", p=P)

            ms = [const_pool.tile([P, K, D], _BF16, name=f"ms{t}")
                  for t in range(NBT)]
            ms_flat = [m.rearrange("p k d -> p (k d)") for m in ms]

            esum = [acc.tile([P, K, NJB], _F32, name=f"esum{t}")
                    for t in range(NBT)]
            esum2 = [acc.tile([P, K, 2], _F32, name=f"esum2{t}")
                     for t in range(NBT)]
            for t in range(NBT):
                nc.gpsimd.memset(esum[t], 0.0)
                nc.gpsimd.memset(esum2[t], 0.0)

            # ---- matmuls: col-chunks, descending; 128-col head chunk so
            # the first pairwise block can start ASAP. Chunks 0+1 share a
            # PSUM bank (disjoint col slices).
            chunks = [(1920, 2048, 0), (1536, 1920, 0), (1024, 1536, 1),
                      (512, 1024, 2), (0, 512, 3)]
            psums = {
                (t, bk): psum_pool.tile(
                    [P, 512], _F32, tag=f"ps{t}_{bk}", name=f"ps{t}_{bk}"
                )
                for t in range(NBT)
                for bk in range(4)
            }
            for c0, c1, bk in chunks:
                w_sb = wpool.tile([P, FB, 512], _BF16, tag="w")
                wv = w_sb[:, :, : c1 - c0]
                nc.sync.dma_start(out=wv, in_=w_r[:, :, c0:c1])
                base = 1536 - bk * 512
                lo, hi = c0 - base, c1 - base
                for t in range(NBT):
                    ps = psums[(t, bk)]
                    for fb in range(FB):
                        nc.tensor.matmul(
                            ps[:, lo:hi],
                            xt_sb[:, fb, t * P : (t + 1) * P],
                            wv[:, fb, :],
                            start=(fb == 0),
                            stop=(fb == FB - 1),
                        )
                    nc.scalar.copy(out=ms_flat[t][:, c0:c1], in_=ps[:, lo:hi])

            # ---- pairwise stage, interleaved over tiles per chunk group --
            # j-blocks grouped by the matmul chunk that completes them:
            # block j0 needs ms cols [16*j0, 2048).
            groups = [
                [120],
                [112, 104, 96],
                [88, 80, 72, 64],
                [56, 48, 40, 32],
                [24, 16, 8, 0],
            ]

            def pairwise_block(t, j0, i0, i1, si, ch):
                """Pairs (i, j) for i in [i0, i1), j in [j0, j0+8)."""
                ni = i1 - i0
                m = ms[t]
                dvf = work.tile([P, 96 * JB * D], _BF16, tag="dv",
                                name="dv")
                dv = dvf[:, : ni * JB * D].rearrange(
                    "p (i j d) -> p i j d", j=JB, d=D
                )
                in0 = m[:, i0:i1, :].unsqueeze(2).broadcast_to([P, ni, JB, D])
                in1 = (
                    m[:, j0 : j0 + JB, :]
                    .unsqueeze(1)
                    .broadcast_to([P, ni, JB, D])
                )
                nc.vector.tensor_sub(dv, in0, in1)
                nc.scalar.activation(
                    out=dv, in_=dv, func=mybir.ActivationFunctionType.Abs
                )
                l1f = mid.tile([P, 96 * JB * 8], _BF16, tag="l1", name="l1")
                l1 = l1f[:, : ni * JB * 8].rearrange(
                    "p (i j d) -> p i j d", j=JB, d=8
                )
                nc.vector.tensor_add(l1, dv[:, :, :, 0:8], dv[:, :, :, 8:16])
                l2f = mid.tile([P, 96 * JB * 4], _BF16, tag="l2", name="l2")
                l2 = l2f[:, : ni * JB * 4].rearrange(
                    "p (i j d) -> p i j d", j=JB, d=4
                )
                nc.vector.tensor_add(l2, l1[:, :, :, 0:4], l1[:, :, :, 4:8])
                l3f = mid.tile([P, 96 * JB * 2], _BF16, tag="l3", name="l3")
                l3 = l3f[:, : ni * JB * 2].rearrange(
                    "p (i j d) -> p i j d", j=JB, d=2
                )
                nc.vector.tensor_add(l3, l2[:, :, :, 0:2], l2[:, :, :, 2:4])
                # exp(-(a+b)) = exp(-a)*exp(-b): skips the 1x final add;
                # the product runs at bf16 2x on DVE instead.
                e3f = mid.tile([P, 2, 96 * JB], _BF16, tag="e3", name="e3")
                for h in range(2):
                    nc.scalar.activation(
                        out=e3f[:, h, : ni * JB],
                        in_=l3[:, :, :, h].rearrange("p i j -> p (i j)"),
                        func=mybir.ActivationFunctionType.Exp,
                        scale=-1.0,
                    )
                eef = mid.tile([P, 96 * JB], _BF16, tag="ee", name="ee")
                ee = eef[:, : ni * JB].rearrange("p (i j) -> p i j", j=JB)
                nc.vector.tensor_mul(
                    eef[:, : ni * JB],
                    e3f[:, 0, : ni * JB],
                    e3f[:, 1, : ni * JB],
                )
                # O[i] += sum_{j in block} E[i, j]
                nc.vector.tensor_reduce(
                    out=esum[t][:, i0:i1, si : si + 1],
                    in_=ee,
                    axis=_AX,
                    op=_ADD,
                )
                # O[j] += sum_{i : i >= j0+8} E[i, j]  (mirror of cross pairs)
                lo = max(i0, j0 + JB)
                if lo < i1:
                    nc.vector.tensor_reduce(
                        out=esum2[t][:, j0 : j0 + JB, ch : ch + 1],
                        in_=ee[:, lo - i0 : ni, :].transpose([0, 2, 1]),
                        axis=_AX,
                        op=_ADD,
                    )

            for gi, group in enumerate(groups):
                for j0 in group:
                    si = j0 // JB
                    for t in range(NBT):
                        if K - j0 > 96:
                            pairwise_block(t, j0, j0, 64, si, 0)
                            pairwise_block(t, j0, 64, K, si, 1)
                        else:
                            pairwise_block(t, j0, j0, K, si, 0)

            # ---- final: O = sum_si esum + mirror contributions ----
            for t in range(NBT):
                o1 = acc.tile([P, K, 1], _F32, name=f"o1_{t}")
                nc.vector.tensor_reduce(
                    out=o1, in_=esum[t], axis=_AX, op=_ADD
                )
                o2 = acc.tile([P, K], _F32, name=f"o2_{t}")
                nc.vector.tensor_add(o2, esum2[t][:, :, 0], esum2[t][:, :, 1])
                o3 = acc.tile([P, K], _F32, name=f"o3_{t}")
                nc.vector.tensor_add(o3, o1[:, :, 0], o2)
                nc.sync.dma_start(out=out[t * P : (t + 1) * P, :], in_=o3)
    nc.compile()
    return nc


_cached = {}


def _get_nc():
    if "nc" not in _cached:
        _cached["nc"] = _build_nc()
    return _cached["nc"]


def kernel(x: np.ndarray, W: np.ndarray) -> np.ndarray:
    nc = _get_nc()
    xt = np.ascontiguousarray(x.T.astype(bfloat16))  # [F, B]
    wb = np.ascontiguousarray(W.astype(bfloat16))    # [F, ND]
    in_maps = [
        {
            "xt": np.ascontiguousarray(xt[:, c * BL : (c + 1) * BL]),
            "w": wb,
        }
        for c in range(NCORES)
    ]
    res = run_bass_kernel_spmd(nc, in_maps, core_ids=list(range(NCORES)))
    return np.concatenate(
        [res.results[c]["out"] for c in range(NCORES)], axis=0
    ).astype(np.float32)


# revision 20
# speedup vs baseline: 1.0203x; 1.0203x over previous
"""MinibatchDiscrimination kernel for Trainium2 (8 NeuronCores, SPMD).

Math: Ms = (x @ W).reshape(B, 128, 16)
      norm[b,i,j] = sum_d |Ms[b,i,d] - Ms[b,j,d]|
      out[b,i]    = sum_j exp(-norm[b,i,j])

Sharding: data-parallel over batch B across 8 cores (256 samples each).
W replicated. Host pre-casts to bf16 and pre-transposes x so the
stationary matmul operand loads without an on-device transpose.

Engine assignment (per 128-sample tile, partitions = batch):
  - PE: Ms = x @ W in 4 col-chunks (descending), both tiles per chunk,
    8 PSUM banks; W streamed per chunk and freed.
  - DVE: broadcast subtract (bf16 2x), tree L1 (bf16 2x), per-block
    row-sum reduce and mirrored (transposed) reduce.
  - ACT: |.| in place on the diff, exp(-norm) -> bf16.
  - Pool (gpsimd): tree L2/L3/L4, PSUM->SBUF Ms copies, esum final
    reduction tree, memsets.
Symmetry: for j-block [j0, j0+8) compute only i in [j0, 128); mirrored
contributions for i >= j0+8 are added to O[j] via a transposed reduce.
Blocks with ni > 64 split into two i-chunks (SBUF + drain latency).
"""

import os
import sys

sys.path.insert(0, "/opt/trn_rl_repo")
os.environ.setdefault("MYCRO_LOCAL_CACHE", "1")

import numpy as np
from ml_dtypes import bfloat16

import concourse.bacc as bacc
import concourse.bass as bass
import concourse.tile as tile
from concourse import mybir
from concourse.bass_utils import run_bass_kernel_spmd

B, F, K, D = 2048, 2048, 128, 16
NCORES = 8
BL = B // NCORES          # 256 rows per core
P = 128                   # partitions
NBT = BL // P             # 2 batch tiles per core
FB = F // P               # 16 contraction blocks
ND = K * D                # 2048 output cols of the matmul
JB = 8                    # j-block width
NJB = K // JB             # 16 j-blocks

_BF16 = mybir.dt.bfloat16
_F32 = mybir.dt.float32
_AX = mybir.AxisListType.X
_ADD = mybir.AluOpType.add


def _build_nc():
    nc = bacc.Bacc("TRN2", target_bir_lowering=False, debug=False)
    xt = nc.dram_tensor("xt", [F, BL], _BF16, kind="ExternalInput")
    w = nc.dram_tensor("w", [F, ND], _BF16, kind="ExternalInput")
    out = nc.dram_tensor("out", [BL, K], _F32, kind="ExternalOutput")

    with tile.TileContext(nc) as tc:
        with (
            tc.tile_pool(name="const", bufs=1) as const_pool,
            tc.tile_pool(name="wstream", bufs=2) as wpool,
            tc.tile_pool(name="work", bufs=3) as work,
            tc.tile_pool(name="mid", bufs=2) as mid,
            tc.tile_pool(name="acc", bufs=1) as acc,
            tc.tile_pool(name="psum", bufs=1, space="PSUM") as psum_pool,
        ):
            xt_sb = const_pool.tile([P, FB, BL], _BF16)
            xt_r = xt.rearrange("(fb p) b -> p fb b", p=P)
            for fb in range(FB):
                nc.gpsimd.dma_start(out=xt_sb[:, fb, :], in_=xt_r[:, fb, :])
            w_r = w.rearrange("(fb p) n -> p fb n", p=P)

            ms = [const_pool.tile([P, K, D], _BF16, name=f"ms{t}")
                  for t in range(NBT)]
            ms_flat = [m.rearrange("p k d -> p (k d)") for m in ms]

            esum = [acc.tile([P, K, NJB], _F32, name=f"esum{t}")
                    for t in range(NBT)]
            esum2 = [acc.tile([P, K, 2], _F32, name=f"esum2{t}")
                     for t in range(NBT)]
            for t in range(NBT):
                nc.gpsimd.memset(esum[t], 0.0)
                nc.gpsimd.memset(esum2[t], 0.0)

            # ---- matmuls: col-chunks, descending; 128-col head chunk so
            # the first pairwise block can start ASAP. Chunks 0+1 share a
            # PSUM bank (disjoint col slices).
            chunks = [(1920, 2048, 0), (1536, 1920, 0), (1024, 1536, 1),
                      (512, 1024, 2), (0, 512, 3)]
            psums = {
                (t, bk): psum_pool.tile(
                    [P, 512], _F32, tag=f"ps{t}_{bk}", name=f"ps{t}_{bk}"
                )
                for t in range(NBT)
                for bk in range(4)
            }
            for c0, c1, bk in chunks:
                w_sb = wpool.tile([P, FB, 512], _BF16, tag="w")
                wv = w_sb[:, :, : c1 - c0]
                nc.sync.dma_start(out=wv, in_=w_r[:, :, c0:c1])
                base = 1536 - bk * 512
                lo, hi = c0 - base, c1 - base
                for t in range(NBT):
                    ps = psums[(t, bk)]
                    for fb in range(FB):
                        nc.tensor.matmul(
                            ps[:, lo:hi],
                            xt_sb[:, fb, t * P : (t + 1) * P],
                            wv[:, fb, :],
                            start=(fb == 0),
                            stop=(fb == FB - 1),
                        )
                    nc.scalar.copy(out=ms_flat[t][:, c0:c1], in_=ps[:, lo:hi])

            # ---- pairwise stage, interleaved over tiles per chunk group --
            # j-blocks grouped by the matmul chunk that completes them:
            # block j0 needs ms cols [16*j0, 2048).
            groups = [
                [120],
                [112, 104, 96],
                [88, 80, 72, 64],
                [56, 48, 40, 32],
                [24, 16, 8, 0],
            ]

            def pairwise_block(t, j0, i0, i1, si, ch):
                """Pairs (i, j) for i in [i0, i1), j in [j0, j0+8)."""
                ni = i1 - i0
                m = ms[t]
                dvf = work.tile([P, 96 * JB * D], _BF16, tag="dv",
                                name="dv")
                dv = dvf[:, : ni * JB * D].rearrange(
                    "p (i j d) -> p i j d", j=JB, d=D
                )
                in0 = m[:, i0:i1, :].unsqueeze(2).broadcast_to([P, ni, JB, D])
                in1 = (
                    m[:, j0 : j0 + JB, :]
                    .unsqueeze(1)
                    .broadcast_to([P, ni, JB, D])
                )
                nc.vector.tensor_sub(dv, in0, in1)
                nc.scalar.activation(
                    out=dv, in_=dv, func=mybir.ActivationFunctionType.Abs
                )
                l1f = mid.tile([P, 96 * JB * 8], _BF16, tag="l1", name="l1")
                l1 = l1f[:, : ni * JB * 8].rearrange(
                    "p (i j d) -> p i j d", j=JB, d=8
                )
                nc.vector.tensor_add(l1, dv[:, :, :, 0:8], dv[:, :, :, 8:16])
                l2f = mid.tile([P, 96 * JB * 4], _BF16, tag="l2", name="l2")
                l2 = l2f[:, : ni * JB * 4].rearrange(
                    "p (i j d) -> p i j d", j=JB, d=4
                )
                nc.vector.tensor_add(l2, l1[:, :, :, 0:4], l1[:, :, :, 4:8])
                l3f = mid.tile([P, 96 * JB * 2], _BF16, tag="l3", name="l3")
                l3 = l3f[:, : ni * JB * 2].rearrange(
                    "p (i j d) -> p i j d", j=JB, d=2
                )
                nc.vector.tensor_add(l3, l2[:, :, :, 0:2], l2[:, :, :, 2:4])
                # exp(-(a+b)) = exp(-a)*exp(-b): skips the 1x final add;
                # the product runs at bf16 2x on DVE instead.
                e3f = mid.tile([P, 2, 96 * JB], _BF16, tag="e3", name="e3")
                for h in range(2):
                    nc.scalar.activation(
                        out=e3f[:, h, : ni * JB],
                        in_=l3[:, :, :, h].rearrange("p i j -> p (i j)"),
                        func=mybir.ActivationFunctionType.Exp,
                        scale=-1.0,
                    )
                eef = mid.tile([P, 96 * JB], _BF16, tag="ee", name="ee")
                ee = eef[:, : ni * JB].rearrange("p (i j) -> p i j", j=JB)
                nc.vector.tensor_mul(
                    eef[:, : ni * JB],
                    e3f[:, 0, : ni * JB],
                    e3f[:, 1, : ni * JB],
                )
                # O[i] += sum_{j in block} E[i, j] — small bf16 add-tree
                # on the otherwise-idle Pool engine (keeps DVE free).
                r1f = mid.tile([P, 96 * 4], _BF16, tag="r1", name="r1")
                r1 = r1f[:, : ni * 4].rearrange("p (i j) -> p i j", j=4)
                nc.gpsimd.tensor_add(r1, ee[:, :, 0:4], ee[:, :, 4:8])
                r2f = mid.tile([P, 96 * 2], _BF16, tag="r2", name="r2")
                r2 = r2f[:, : ni * 2].rearrange("p (i j) -> p i j", j=2)
                nc.gpsimd.tensor_add(r2, r1[:, :, 0:2], r1[:, :, 2:4])
                nc.gpsimd.tensor_add(
                    esum[t][:, i0:i1, si], r2[:, :, 0], r2[:, :, 1]
                )
                # O[j] += sum_{i : i >= j0+8} E[i, j]  (mirror of cross pairs)
                lo = max(i0, j0 + JB)
                if lo < i1:
                    nc.vector.tensor_reduce(
                        out=esum2[t][:, j0 : j0 + JB, ch : ch + 1],
                        in_=ee[:, lo - i0 : ni, :].transpose([0, 2, 1]),
                        axis=_AX,
                        op=_ADD,
                    )

            for gi, group in enumerate(groups):
                for j0 in group:
                    si = j0 // JB
                    for t in range(NBT):
                        if K - j0 > 96:
                            pairwise_block(t, j0, j0, 64, si, 0)
                            pairwise_block(t, j0, 64, K, si, 1)
                        else:
                            pairwise_block(t, j0, j0, K, si, 0)

            # ---- final: O = sum_si esum + mirror contributions ----
            for t in range(NBT):
                o1 = acc.tile([P, K, 1], _F32, name=f"o1_{t}")
                nc.vector.tensor_reduce(
                    out=o1, in_=esum[t], axis=_AX, op=_ADD
                )
                o2 = acc.tile([P, K], _F32, name=f"o2_{t}")
                nc.vector.tensor_add(o2, esum2[t][:, :, 0], esum2[t][:, :, 1])
                o3 = acc.tile([P, K], _F32, name=f"o3_{t}")
                nc.vector.tensor_add(o3, o1[:, :, 0], o2)
                nc.sync.dma_start(out=out[t * P : (t + 1) * P, :], in_=o3)
    nc.compile()
    return nc


_cached = {}


def _get_nc():
    if "nc" not in _cached:
        _cached["nc"] = _build_nc()
    return _cached["nc"]


def kernel(x: np.ndarray, W: np.ndarray) -> np.ndarray:
    nc = _get_nc()
    xt = np.ascontiguousarray(x.T.astype(bfloat16))  # [F, B]
    wb = np.ascontiguousarray(W.astype(bfloat16))    # [F, ND]
    in_maps = [
        {
            "xt": np.ascontiguousarray(xt[:, c * BL : (c + 1) * BL]),
            "w": wb,
        }
        for c in range(NCORES)
    ]
    res = run_bass_kernel_spmd(nc, in_maps, core_ids=list(range(NCORES)))
    return np.concatenate(
        [res.results[c]["out"] for c in range(NCORES)], axis=0
    ).astype(np.float32)


# revision 21
# speedup vs baseline: 1.0484x; 1.0275x over previous
"""MinibatchDiscrimination kernel for Trainium2 (8 NeuronCores, SPMD).

Math: Ms = (x @ W).reshape(B, 128, 16)
      norm[b,i,j] = sum_d |Ms[b,i,d] - Ms[b,j,d]|
      out[b,i]    = sum_j exp(-norm[b,i,j])

Sharding: data-parallel over batch B across 8 cores (256 samples each).
W replicated. Host pre-casts to bf16 and pre-transposes x so the
stationary matmul operand loads without an on-device transpose.

Engine assignment (per 128-sample tile, partitions = batch):
  - PE: Ms = x @ W in col-chunks (descending, 128-col head chunk so the
    first pairwise block starts early), both tiles per chunk, 8 PSUM
    banks; W streamed per chunk and freed.
  - DVE: broadcast subtract (bf16 2x), add-tree L1..L3 (bf16 2x), the
    exp-product multiply, and the mirrored (transposed) reduce. DVE is
    the bottleneck engine (~90% busy).
  - ACT: |.| in place on the diff, then exp applied to the two l3
    halves separately -- exp(-(a+b)) = exp(-a)*exp(-b) -- so the final
    tree level becomes a 2x bf16 DVE multiply instead of a 1x fp32 add.
    Also does PSUM->SBUF Ms copies.
  - Pool (gpsimd): per-block row-sum bf16 add-tree into esum (cheap
    enough at ~50% duty not to stretch DVE; bulk tree stages are NOT
    viable here -- Pool costs ~1.5us/instruction), plus memsets.
Symmetry: for j-block [j0, j0+8) compute only i in [j0, 128); mirrored
contributions for i >= j0+8 are added to O[j] via a transposed reduce.
Blocks with ni > 96 split into two i-chunks (SBUF + drain latency).
"""

import os
import sys

sys.path.insert(0, "/opt/trn_rl_repo")
os.environ.setdefault("MYCRO_LOCAL_CACHE", "1")

import numpy as np
from ml_dtypes import bfloat16

import concourse.bacc as bacc
import concourse.bass as bass
import concourse.tile as tile
from concourse import mybir
from concourse.bass_utils import run_bass_kernel_spmd

B, F, K, D = 2048, 2048, 128, 16
NCORES = 8
BL = B // NCORES          # 256 rows per core
P = 128                   # partitions
NBT = BL // P             # 2 batch tiles per core
FB = F // P               # 16 contraction blocks
ND = K * D                # 2048 output cols of the matmul
JB = 8                    # j-block width
NJB = K // JB             # 16 j-blocks

_BF16 = mybir.dt.bfloat16
_F32 = mybir.dt.float32
_AX = mybir.AxisListType.X
_ADD = mybir.AluOpType.add


def _build_nc():
    nc = bacc.Bacc("TRN2", target_bir_lowering=False, debug=False)
    xt = nc.dram_tensor("xt", [F, BL], _BF16, kind="ExternalInput")
    w = nc.dram_tensor("w", [F, ND], _BF16, kind="ExternalInput")
    out = nc.dram_tensor("out", [BL, K], _F32, kind="ExternalOutput")

    with tile.TileContext(nc) as tc:
        with (
            tc.tile_pool(name="const", bufs=1) as const_pool,
            tc.tile_pool(name="wstream", bufs=2) as wpool,
            tc.tile_pool(name="work", bufs=3) as work,
            tc.tile_pool(name="mid", bufs=2) as mid,
            tc.tile_pool(name="acc", bufs=1) as acc,
            tc.tile_pool(name="psum", bufs=1, space="PSUM") as psum_pool,
        ):
            xt_sb = const_pool.tile([P, FB, BL], _BF16)
            xt_r = xt.rearrange("(fb p) b -> p fb b", p=P)
            for fb in range(FB):
                nc.gpsimd.dma_start(out=xt_sb[:, fb, :], in_=xt_r[:, fb, :])
            w_r = w.rearrange("(fb p) n -> p fb n", p=P)

            ms = [const_pool.tile([P, K, D], _BF16, name=f"ms{t}")
                  for t in range(NBT)]
            ms_flat = [m.rearrange("p k d -> p (k d)") for m in ms]

            esum = [acc.tile([P, K, NJB], _F32, name=f"esum{t}")
                    for t in range(NBT)]
            esum2 = [acc.tile([P, K, 2], _F32, name=f"esum2{t}")
                     for t in range(NBT)]
            for t in range(NBT):
                nc.gpsimd.memset(esum[t], 0.0)
                nc.gpsimd.memset(esum2[t], 0.0)

            # ---- matmuls: col-chunks, descending; 128-col head chunk so
            # the first pairwise block can start ASAP. Chunks 0+1 share a
            # PSUM bank (disjoint col slices).
            chunks = [(1920, 2048, 0), (1536, 1920, 0), (1024, 1536, 1),
                      (512, 1024, 2), (0, 512, 3)]
            psums = {
                (t, bk): psum_pool.tile(
                    [P, 512], _F32, tag=f"ps{t}_{bk}", name=f"ps{t}_{bk}"
                )
                for t in range(NBT)
                for bk in range(4)
            }
            for c0, c1, bk in chunks:
                w_sb = wpool.tile([P, FB, 512], _BF16, tag="w")
                wv = w_sb[:, :, : c1 - c0]
                nc.sync.dma_start(out=wv, in_=w_r[:, :, c0:c1])
                base = 1536 - bk * 512
                lo, hi = c0 - base, c1 - base
                for t in range(NBT):
                    ps = psums[(t, bk)]
                    for fb in range(FB):
                        nc.tensor.matmul(
                            ps[:, lo:hi],
                            xt_sb[:, fb, t * P : (t + 1) * P],
                            wv[:, fb, :],
                            start=(fb == 0),
                            stop=(fb == FB - 1),
                        )
                    nc.scalar.copy(out=ms_flat[t][:, c0:c1], in_=ps[:, lo:hi])

            # ---- pairwise stage, interleaved over tiles per chunk group --
            # j-blocks grouped by the matmul chunk that completes them:
            # block j0 needs ms cols [16*j0, 2048).
            groups = [
                [120],
                [112, 104, 96],
                [88, 80, 72, 64],
                [56, 48, 40, 32],
                [24, 16, 8, 0],
            ]

            def pairwise_block(t, j0, i0, i1, si, ch):
                """Pairs (i, j) for i in [i0, i1), j in [j0, j0+8)."""
                ni = i1 - i0
                m = ms[t]
                dvf = work.tile([P, 96 * JB * D], _BF16, tag="dv",
                                name="dv")
                dv = dvf[:, : ni * JB * D].rearrange(
                    "p (i j d) -> p i j d", j=JB, d=D
                )
                in0 = m[:, i0:i1, :].unsqueeze(2).broadcast_to([P, ni, JB, D])
                in1 = (
                    m[:, j0 : j0 + JB, :]
                    .unsqueeze(1)
                    .broadcast_to([P, ni, JB, D])
                )
                nc.vector.tensor_sub(dv, in0, in1)
                nc.scalar.activation(
                    out=dv, in_=dv, func=mybir.ActivationFunctionType.Abs
                )
                l1f = mid.tile([P, 96 * JB * 8], _BF16, tag="l1", name="l1")
                l1 = l1f[:, : ni * JB * 8].rearrange(
                    "p (i j d) -> p i j d", j=JB, d=8
                )
                nc.vector.tensor_add(l1, dv[:, :, :, 0:8], dv[:, :, :, 8:16])
                l2f = mid.tile([P, 96 * JB * 4], _BF16, tag="l2", name="l2")
                l2 = l2f[:, : ni * JB * 4].rearrange(
                    "p (i j d) -> p i j d", j=JB, d=4
                )
                nc.vector.tensor_add(l2, l1[:, :, :, 0:4], l1[:, :, :, 4:8])
                l3f = mid.tile([P, 96 * JB * 2], _BF16, tag="l3", name="l3")
                l3 = l3f[:, : ni * JB * 2].rearrange(
                    "p (i j d) -> p i j d", j=JB, d=2
                )
                nc.vector.tensor_add(l3, l2[:, :, :, 0:2], l2[:, :, :, 2:4])
                # exp(-(a+b)) = exp(-a)*exp(-b): skips the 1x final add;
                # the product runs at bf16 2x on DVE instead.
                e3f = mid.tile([P, 2, 96 * JB], _BF16, tag="e3", name="e3")
                for h in range(2):
                    nc.scalar.activation(
                        out=e3f[:, h, : ni * JB],
                        in_=l3[:, :, :, h].rearrange("p i j -> p (i j)"),
                        func=mybir.ActivationFunctionType.Exp,
                        scale=-1.0,
                    )
                eef = mid.tile([P, 96 * JB], _BF16, tag="ee", name="ee")
                ee = eef[:, : ni * JB].rearrange("p (i j) -> p i j", j=JB)
                nc.vector.tensor_mul(
                    eef[:, : ni * JB],
                    e3f[:, 0, : ni * JB],
                    e3f[:, 1, : ni * JB],
                )
                # O[i] += sum_{j in block} E[i, j] — small bf16 add-tree
                # on the otherwise-idle Pool engine (keeps DVE free).
                r1f = mid.tile([P, 96 * 4], _BF16, tag="r1", name="r1")
                r1 = r1f[:, : ni * 4].rearrange("p (i j) -> p i j", j=4)
                nc.gpsimd.tensor_add(r1, ee[:, :, 0:4], ee[:, :, 4:8])
                r2f = mid.tile([P, 96 * 2], _BF16, tag="r2", name="r2")
                r2 = r2f[:, : ni * 2].rearrange("p (i j) -> p i j", j=2)
                nc.gpsimd.tensor_add(r2, r1[:, :, 0:2], r1[:, :, 2:4])
                nc.gpsimd.tensor_add(
                    esum[t][:, i0:i1, si], r2[:, :, 0], r2[:, :, 1]
                )
                # O[j] += sum_{i : i >= j0+8} E[i, j]  (mirror of cross pairs)
                lo = max(i0, j0 + JB)
                if lo < i1:
                    nc.vector.tensor_reduce(
                        out=esum2[t][:, j0 : j0 + JB, ch : ch + 1],
                        in_=ee[:, lo - i0 : ni, :].transpose([0, 2, 1]),
                        axis=_AX,
                        op=_ADD,
                    )

            for gi, group in enumerate(groups):
                for j0 in group:
                    si = j0 // JB
                    for t in range(NBT):
                        if K - j0 > 96:
                            pairwise_block(t, j0, j0, 64, si, 0)
                            pairwise_block(t, j0, 64, K, si, 1)
                        else:
                            pairwise_block(t, j0, j0, K, si, 0)

            # ---- final: O = sum_si esum + mirror contributions ----
            for t in range(NBT):
                o1 = acc.tile([P, K, 1], _F32, name=f"o1_{t}")
                nc.vector.tensor_reduce(
                    out=o1, in_=esum[t], axis=_AX, op=_ADD
                )
                o2 = acc.tile([P, K], _F32, name=f"o2_{t}")
                nc.vector.tensor_add(o2, esum2[t][:, :, 0], esum2[t][:, :, 1])
                o3 = acc.tile([P, K], _F32, name=f"o3_{t}")
                nc.vector.tensor_add(o3, o1[:, :, 0], o2)
                nc.sync.dma_start(out=out[t * P : (t + 1) * P, :], in_=o3)
    nc.compile()
    return nc


_cached = {}


def _get_nc():
    if "nc" not in _cached:
        _cached["nc"] = _build_nc()
    return _cached["nc"]


def kernel(x: np.ndarray, W: np.ndarray) -> np.ndarray:
    nc = _get_nc()
    xt = np.ascontiguousarray(x.T.astype(bfloat16))  # [F, B]
    wb = np.ascontiguousarray(W.astype(bfloat16))    # [F, ND]
    in_maps = [
        {
            "xt": np.ascontiguousarray(xt[:, c * BL : (c + 1) * BL]),
            "w": wb,
        }
        for c in range(NCORES)
    ]
    res = run_bass_kernel_spmd(nc, in_maps, core_ids=list(range(NCORES)))
    return np.concatenate(
        [res.results[c]["out"] for c in range(NCORES)], axis=0
    ).astype(np.float32)


# revision 22
# speedup vs baseline: 1.0490x; 1.0006x over previous
"""MinibatchDiscrimination kernel for Trainium2 (8 NeuronCores, SPMD).

Math: Ms = (x @ W).reshape(B, 128, 16)
      norm[b,i,j] = sum_d |Ms[b,i,d] - Ms[b,j,d]|
      out[b,i]    = sum_j exp(-norm[b,i,j])

Sharding: data-parallel over batch B across 8 cores (256 samples each).
W replicated. Host pre-casts to bf16 and pre-transposes x so the
stationary matmul operand loads without an on-device transpose.

Engine assignment (per 128-sample tile, partitions = batch):
  - PE: Ms = x @ W in col-chunks (descending, 128-col head chunk so the
    first pairwise block starts early), both tiles per chunk, 8 PSUM
    banks; W streamed per chunk and freed.
  - DVE: broadcast subtract (bf16 2x), add-tree L1..L3 (bf16 2x), the
    exp-product multiply, and the mirrored (transposed) reduce. DVE is
    the bottleneck engine (~90% busy).
  - ACT: |.| in place on the diff, then exp applied to the two l3
    halves separately -- exp(-(a+b)) = exp(-a)*exp(-b) -- so the final
    tree level becomes a 2x bf16 DVE multiply instead of a 1x fp32 add.
    Also does PSUM->SBUF Ms copies.
  - Pool (gpsimd): per-block row-sum bf16 add-tree into esum (cheap
    enough at ~50% duty not to stretch DVE; bulk tree stages are NOT
    viable here -- Pool costs ~1.5us/instruction), plus memsets.
Symmetry: for j-block [j0, j0+8) compute only i in [j0, 128); mirrored
contributions for i >= j0+8 are added to O[j] via a transposed reduce.
Blocks with ni > 96 split into two i-chunks (SBUF + drain latency).
"""

import os
import sys

sys.path.insert(0, "/opt/trn_rl_repo")
os.environ.setdefault("MYCRO_LOCAL_CACHE", "1")

import numpy as np
from ml_dtypes import bfloat16

import concourse.bacc as bacc
import concourse.bass as bass
import concourse.tile as tile
from concourse import mybir
from concourse.bass_utils import run_bass_kernel_spmd

B, F, K, D = 2048, 2048, 128, 16
NCORES = 8
BL = B // NCORES          # 256 rows per core
P = 128                   # partitions
NBT = BL // P             # 2 batch tiles per core
FB = F // P               # 16 contraction blocks
ND = K * D                # 2048 output cols of the matmul
JB = 8                    # j-block width
NJB = K // JB             # 16 j-blocks

_BF16 = mybir.dt.bfloat16
_F32 = mybir.dt.float32
_AX = mybir.AxisListType.X
_ADD = mybir.AluOpType.add


def _build_nc():
    nc = bacc.Bacc("TRN2", target_bir_lowering=False, debug=False)
    xt = nc.dram_tensor("xt", [F, BL], _BF16, kind="ExternalInput")
    w = nc.dram_tensor("w", [F, ND], _BF16, kind="ExternalInput")
    out = nc.dram_tensor("out", [BL, K], _F32, kind="ExternalOutput")

    with tile.TileContext(nc) as tc:
        with (
            tc.tile_pool(name="const", bufs=1) as const_pool,
            tc.tile_pool(name="wstream", bufs=2) as wpool,
            tc.tile_pool(name="work", bufs=3) as work,
            tc.tile_pool(name="mid", bufs=2) as mid,
            tc.tile_pool(name="acc", bufs=1) as acc,
            tc.tile_pool(name="psum", bufs=1, space="PSUM") as psum_pool,
        ):
            xt_sb = const_pool.tile([P, FB, BL], _BF16)
            xt_r = xt.rearrange("(fb p) b -> p fb b", p=P)
            nc.scalar.dma_start(out=xt_sb, in_=xt_r)
            w_r = w.rearrange("(fb p) n -> p fb n", p=P)

            ms = [const_pool.tile([P, K, D], _BF16, name=f"ms{t}")
                  for t in range(NBT)]
            ms_flat = [m.rearrange("p k d -> p (k d)") for m in ms]

            esum = [acc.tile([P, K, NJB], _F32, name=f"esum{t}")
                    for t in range(NBT)]
            esum2 = [acc.tile([P, K, 2], _F32, name=f"esum2{t}")
                     for t in range(NBT)]
            for t in range(NBT):
                nc.gpsimd.memset(esum[t], 0.0)
                nc.gpsimd.memset(esum2[t], 0.0)

            # ---- matmuls: col-chunks, descending; 128-col head chunk so
            # the first pairwise block can start ASAP. Chunks 0+1 share a
            # PSUM bank (disjoint col slices).
            chunks = [(1920, 2048, 0), (1536, 1920, 0), (1024, 1536, 1),
                      (512, 1024, 2), (0, 512, 3)]
            psums = {
                (t, bk): psum_pool.tile(
                    [P, 512], _F32, tag=f"ps{t}_{bk}", name=f"ps{t}_{bk}"
                )
                for t in range(NBT)
                for bk in range(4)
            }
            for c0, c1, bk in chunks:
                w_sb = wpool.tile([P, FB, 512], _BF16, tag="w")
                wv = w_sb[:, :, : c1 - c0]
                nc.sync.dma_start(out=wv, in_=w_r[:, :, c0:c1])
                base = 1536 - bk * 512
                lo, hi = c0 - base, c1 - base
                for t in range(NBT):
                    ps = psums[(t, bk)]
                    for fb in range(FB):
                        nc.tensor.matmul(
                            ps[:, lo:hi],
                            xt_sb[:, fb, t * P : (t + 1) * P],
                            wv[:, fb, :],
                            start=(fb == 0),
                            stop=(fb == FB - 1),
                        )
                    nc.scalar.copy(out=ms_flat[t][:, c0:c1], in_=ps[:, lo:hi])

            # ---- pairwise stage, interleaved over tiles per chunk group --
            # j-blocks grouped by the matmul chunk that completes them:
            # block j0 needs ms cols [16*j0, 2048).
            groups = [
                [120],
                [112, 104, 96],
                [88, 80, 72, 64],
                [56, 48, 40, 32],
                [0, 8, 16, 24],
            ]

            def pairwise_block(t, j0, i0, i1, si, ch, rs_dve=False):
                """Pairs (i, j) for i in [i0, i1), j in [j0, j0+8)."""
                ni = i1 - i0
                m = ms[t]
                dvf = work.tile([P, 96 * JB * D], _BF16, tag="dv",
                                name="dv")
                dv = dvf[:, : ni * JB * D].rearrange(
                    "p (i j d) -> p i j d", j=JB, d=D
                )
                in0 = m[:, i0:i1, :].unsqueeze(2).broadcast_to([P, ni, JB, D])
                in1 = (
                    m[:, j0 : j0 + JB, :]
                    .unsqueeze(1)
                    .broadcast_to([P, ni, JB, D])
                )
                nc.vector.tensor_sub(dv, in0, in1)
                nc.scalar.activation(
                    out=dv, in_=dv, func=mybir.ActivationFunctionType.Abs
                )
                l1f = mid.tile([P, 96 * JB * 8], _BF16, tag="l1", name="l1")
                l1 = l1f[:, : ni * JB * 8].rearrange(
                    "p (i j d) -> p i j d", j=JB, d=8
                )
                nc.vector.tensor_add(l1, dv[:, :, :, 0:8], dv[:, :, :, 8:16])
                l2f = mid.tile([P, 96 * JB * 4], _BF16, tag="l2", name="l2")
                l2 = l2f[:, : ni * JB * 4].rearrange(
                    "p (i j d) -> p i j d", j=JB, d=4
                )
                nc.vector.tensor_add(l2, l1[:, :, :, 0:4], l1[:, :, :, 4:8])
                l3f = mid.tile([P, 96 * JB * 2], _BF16, tag="l3", name="l3")
                l3 = l3f[:, : ni * JB * 2].rearrange(
                    "p (i j d) -> p i j d", j=JB, d=2
                )
                nc.vector.tensor_add(l3, l2[:, :, :, 0:2], l2[:, :, :, 2:4])
                # exp(-(a+b)) = exp(-a)*exp(-b): skips the 1x final add;
                # the product runs at bf16 2x on DVE instead.
                e3f = mid.tile([P, 2, 96 * JB], _BF16, tag="e3", name="e3")
                for h in range(2):
                    nc.scalar.activation(
                        out=e3f[:, h, : ni * JB],
                        in_=l3[:, :, :, h].rearrange("p i j -> p (i j)"),
                        func=mybir.ActivationFunctionType.Exp,
                        scale=-1.0,
                    )
                eef = mid.tile([P, 96 * JB], _BF16, tag="ee", name="ee")
                ee = eef[:, : ni * JB].rearrange("p (i j) -> p i j", j=JB)
                nc.vector.tensor_mul(
                    eef[:, : ni * JB],
                    e3f[:, 0, : ni * JB],
                    e3f[:, 1, : ni * JB],
                )
                # O[i] += sum_{j in block} E[i, j] — small bf16 add-tree
                # on the otherwise-idle Pool engine (keeps DVE free).
                # Final chunks reduce on DVE instead: DVE idles in the
                # drain and Pool's ~1.5us/instr would lengthen the tail.
                if rs_dve:
                    nc.vector.tensor_reduce(
                        out=esum[t][:, i0:i1, si : si + 1],
                        in_=ee,
                        axis=_AX,
                        op=_ADD,
                    )
                    return
                r1f = mid.tile([P, 96 * 4], _BF16, tag="r1", name="r1")
                r1 = r1f[:, : ni * 4].rearrange("p (i j) -> p i j", j=4)
                nc.gpsimd.tensor_add(r1, ee[:, :, 0:4], ee[:, :, 4:8])
                r2f = mid.tile([P, 96 * 2], _BF16, tag="r2", name="r2")
                r2 = r2f[:, : ni * 2].rearrange("p (i j) -> p i j", j=2)
                nc.gpsimd.tensor_add(r2, r1[:, :, 0:2], r1[:, :, 2:4])
                nc.gpsimd.tensor_add(
                    esum[t][:, i0:i1, si], r2[:, :, 0], r2[:, :, 1]
                )
                # O[j] += sum_{i : i >= j0+8} E[i, j]  (mirror of cross pairs)
                lo = max(i0, j0 + JB)
                if lo < i1:
                    nc.vector.tensor_reduce(
                        out=esum2[t][:, j0 : j0 + JB, ch : ch + 1],
                        in_=ee[:, lo - i0 : ni, :].transpose([0, 2, 1]),
                        axis=_AX,
                        op=_ADD,
                    )

            for gi, group in enumerate(groups):
                for j0 in group:
                    si = j0 // JB
                    for t in range(NBT):
                        last = gi == 4 and t == NBT - 1 and j0 == 24
                        if K - j0 > 96:
                            pairwise_block(t, j0, j0, 64, si, 0,
                                           rs_dve=last)
                            pairwise_block(t, j0, 64, K, si, 1,
                                           rs_dve=last)
                        else:
                            pairwise_block(t, j0, j0, K, si, 0,
                                           rs_dve=last)

            # ---- final: O = sum_si esum + mirror contributions ----
            for t in range(NBT):
                o1 = acc.tile([P, K, 1], _F32, name=f"o1_{t}")
                nc.vector.tensor_reduce(
                    out=o1, in_=esum[t], axis=_AX, op=_ADD
                )
                o2 = acc.tile([P, K], _F32, name=f"o2_{t}")
                nc.vector.tensor_add(o2, esum2[t][:, :, 0], esum2[t][:, :, 1])
                o3 = acc.tile([P, K], _F32, name=f"o3_{t}")
                nc.vector.tensor_add(o3, o1[:, :, 0], o2)
                nc.sync.dma_start(out=out[t * P : (t + 1) * P, :], in_=o3)
    nc.compile()
    return nc


_cached = {}


def _get_nc():
    if "nc" not in _cached:
        _cached["nc"] = _build_nc()
    return _cached["nc"]


def kernel(x: np.ndarray, W: np.ndarray) -> np.ndarray:
    nc = _get_nc()
    xt = np.ascontiguousarray(x.T.astype(bfloat16))  # [F, B]
    wb = np.ascontiguousarray(W.astype(bfloat16))    # [F, ND]
    in_maps = [
        {
            "xt": np.ascontiguousarray(xt[:, c * BL : (c + 1) * BL]),
            "w": wb,
        }
        for c in range(NCORES)
    ]
    res = run_bass_kernel_spmd(nc, in_maps, core_ids=list(range(NCORES)))
    return np.concatenate(
        [res.results[c]["out"] for c in range(NCORES)], axis=0
    ).astype(np.float32)
